# revision 38
# baseline (speedup 1.0000x reference)
"""Trainium2 Bass kernel for nn_DenoisingModule (non-local attention block).

Reference computation (per batch element n, with C=256 channels, HW=4096):
    theta = W_t x + b_t            # queries  [C, HW]
    phi   = W_p x + b_p            # keys     [C, HW]
    g     = x                      # values   [C, HW]
    S     = theta^T phi / sqrt(C)  # [HW, HW]
    A     = softmax(S, axis=keys)
    f     = g A^T                  # [C, HW]
    out   = x + W_c f + b_c

Sharding: 8 cores; each of the N=4 batch elements is split across 2 cores
by query position (2048 queries per core). Every core holds the full key
set for its batch element, so no collectives are needed.

Per-core device program (SPMD, identical on all cores, data differs):
  - ALL matmuls run in fp8e4 DoubleRow mode (2 contraction k-tiles per
    instruction, 2 fp8 MACs/cell/cycle): the 256-channel contractions
    (projections, scores, conv) are one DR matmul each; the 4096-key PV
    contraction is 16 DR matmuls per 512-query group.
  - scores are computed TRANSPOSED (S^T[k, q] = phi^T theta) so the exp
    output E^T feeds the PV matmul directly as the fp8 moving operand.
  - softmax uses exp(s - 3) (constant shift, softmax-invariant) so the
    max exp value ~96 stays under the TRN fp8e4 max of 240.
  - the softmax denominator is split between engines to balance per-qp
    budgets: even exp pairs are a ones-column DR matmul accumulated in
    PSUM (PE), odd pairs are pairwise adds into an f32r accumulator
    (DVE), folded together by one ones-row matmul per group.
  - softmax normalization is applied AFTER the conv (it commutes with
    the channel mixing): the PV accumulators are released by a plain
    1/64 eviction the moment a group's PV finishes, so the PE never
    waits on the reciprocal (ln/exp on ACT) chain; the reciprocal is
    issued behind the NEXT group's first exp in ACT's in-order queue.
  - scale management for fp8: host prescales W by 16 (out of the fp8
    subnormal range), theta/phi evictions divide by 16; f is evicted as
    ps_f/64; the output stage multiplies by 4/sum (broadcast via a
    ones-column matmul) and adds the f16 residual (+bias host-folded).
  - the activation-table pass is patched so Ln and Exp share one table
    set (the default mapping reloads the ACT table RAMs per group).
  - projection evictions rotate through 4 PSUM buffers alternating
    between ACT and DVE; phi projections for key groups 1-2 interleave
    into attention group 0's pipeline while the xk chunks stream in.
  - input DMAs are issued in first-use order with xt/xqb deferred behind
    xk chunks, all host-pre-shuffled to partition-major >=2KB-contiguous
    layouts; outputs store per 512-query group as soon as they finalize.
  - dummy 1x1 matmuls during the input DMA warm the PE HAM clock gate
    (1.2 -> 2.4 GHz) before the first projection runs.

Toolchain constraint that shapes this file: every TPB engine instruction
(and every DMA) may carry at most ONE semaphore wait, so cross-engine
fan-in is funneled through per-engine collector chains, persistent ring
tiles replace rotating tile pools, and DMA-gated work sits behind
single-purpose barrier matmuls.

The host wrapper rolls x columns per-core so queries are always columns
[0, P) of the local key matrix (keeps the program identical across cores),
and pre-transposes x (and the weight matrices) since the PV matmul needs
x^T as the stationary operand.
"""

import math

import numpy as np

import concourse.bass as bass
import concourse.mybir as mybir
from concourse import bacc
from concourse.bass_utils import run_bass_kernel_spmd
from concourse.tile import TileContext, add_dep_helper


def _combined_ln_exp_tables(orig_fn):
    """Activation-table view that resolves Exp, Ln, Copy (and friends) to
    the single combined `natural_log_exp_and_others` set.

    The compiler's table-load pass picks the FIRST set containing each
    activation function; by default Exp resolves to `exp_and_others` and
    Ln to `natural_log`, so a kernel alternating exp and ln reloads the
    ACT table RAMs (~1.3us each) at every alternation. Hiding those
    functions from every other set makes all of this kernel's activations
    resolve to one set -> exactly one table load. Set ids keep their
    act_info.json positions, so the emitted act_func_set_id stays valid.
    """
    keep = "natural_log_exp_and_others"
    AFT = mybir.ActivationFunctionType
    hide = {AFT.Exp, AFT.Ln, AFT.Identity, AFT.Copy, AFT.MemsetZero}

    def patched(arch):
        tabs = orig_fn(arch)
        return {
            name: (funcs if name == keep else funcs - hide)
            for name, funcs in tabs.items()
        }

    return patched

N, C, H, W = 4, 256, 64, 64
HW = H * W
NCORES = 8
CORES_PER_N = NCORES // N
P_CORE = HW // CORES_PER_N  # queries per core

F32 = mybir.dt.float32
F16 = mybir.dt.float16
FP8 = mybir.dt.float8e4
DR = mybir.MatmulPerfMode.DoubleRow

EXP_SHIFT = -3.0  # softmax-invariant shift keeping exp under fp8e4 max 240
W_SCALE = 16.0    # host premultiplies weights (fp8 subnormal avoidance)
F_DIV = 64.0      # unnormalized f eviction divisor (fp8 range)


def build_program(P, Q, Cc=C):
    """Build the per-core Bass program.

    P: queries handled by this core (first P columns of xk)
    Q: total key positions
    """
    assert P % 512 == 0 and Q % 512 == 0 and Cc == 256
    CT = Cc // 128  # = 2: one DoubleRow pair
    QT = Q // 128
    PG = P // 512
    QG = Q // 512
    scale = float(Cc) ** -0.5

    # All inputs are pre-shuffled on the host into the SBUF layout
    # (partition-major, >=2KB contiguous per partition) so every load DMA
    # uses large descriptors instead of 256B-1KB strided runs.
    NCHUNK = 4
    QC = Q // NCHUNK
    CT_ = Cc // 128
    nc = bacc.Bacc("TRN2", target_bir_lowering=False)
    xk = nc.declare_dram_parameter(
        "xk", [NCHUNK, 128, CT_, QC], FP8, isOutput=False)[:]
    xqb = nc.declare_dram_parameter(
        "xqb", [128, CT_, P], F16, isOutput=False)[:]
    xt = nc.declare_dram_parameter(
        "xt", [128, Q // 128, Cc], FP8, isOutput=False)[:]
    wcat = nc.declare_dram_parameter(
        "wcat", [128, 3, CT_, Cc], FP8, isOutput=False)[:]
    bcat = nc.declare_dram_parameter(
        "bcat", [128, 3, CT_], F32, isOutput=False)[:]
    out = nc.declare_dram_parameter("out", [Cc, P], F16, isOutput=True)[:]

    add = mybir.AluOpType.add
    mult = mybir.AluOpType.mult

    with TileContext(nc) as tc:
        with (
            tc.tile_pool(name="const", bufs=1) as const,
            tc.tile_pool(name="big", bufs=1) as big,
            tc.tile_pool(name="pss", bufs=1, space="PSUM") as pss,
            tc.tile_pool(name="psf", bufs=1, space="PSUM") as psf,
            tc.tile_pool(name="pso", bufs=1, space="PSUM") as pso,
        ):
            QH = Q // 2

            # ---- input loads. Issue order matters: the sync queue drains
            # roughly in order, so the first projection's inputs (xk chunk 0
            # + weights) go first; xt/xqb are gated on the xk chunks so the
            # early transfers get full HBM bandwidth.
            xk_sb = big.tile([128, NCHUNK, CT, QC], FP8, tag="xk")

            def xk_load(h):
                return nc.sync.dma_start(out=xk_sb[:, h], in_=xk[h])

            xk_loads = [xk_load(0)]
            ws_sb = const.tile([128, 3, CT, Cc], FP8, tag="ws")
            w_load = nc.sync.dma_start(out=ws_sb, in_=wcat)
            bb = const.tile([128, 3, CT], F32, tag="bb")
            b_load = nc.sync.dma_start(out=bb, in_=bcat)
            xk_loads.append(xk_load(1))
            xt_sb = big.tile([128, QT, Cc], FP8, tag="xt")
            xt_load = nc.sync.dma_start(out=xt_sb, in_=xt)
            add_dep_helper(xt_load.ins, xk_loads[1].ins, True, "defer xt dma")
            xk_loads += [xk_load(h) for h in range(2, NCHUNK)]
            xq_sb = big.tile([128, CT, P], F16, tag="xq")
            xq_load = nc.sync.dma_start(out=xq_sb, in_=xqb)
            add_dep_helper(xq_load.ins, xk_loads[3].ins, True, "defer xqb dma")

            # persistent tiles (deliberately NOT pool-rotated: pool-slot
            # releases fan in multiple procs; rings keep wait fan-in low)
            acc = big.tile([128, 512], mybir.dt.float32r, tag="acc")
            tsum = big.tile([128, 512], mybir.dt.float32r, tag="tsum")
            th_sb = big.tile([128, CT, P], FP8, tag="th")
            ph_sb = big.tile([128, CT, Q], FP8, tag="ph")
            f_sb = big.tile([128, CT, P], FP8, tag="f")
            e_ring = big.tile([128, 4, 2, 512], FP8, tag="ering")
            rc_ring = const.tile([1, PG, 512], F16, tag="rcring")
            lns = const.tile([1, PG, 512], F32, tag="lns")
            bc_ring = big.tile([128, PG, 512], F32, tag="bcring")
            o_ring = big.tile([128, CT, PG, 512], F16, tag="oring")

            # ---- engine program-order chains + wait collectors ----
            last = {}

            def chain(eng, inst):
                # ordering edges disabled: Bacc legalizes multi-waits, so the
                # Tile scheduler is free to interleave within each engine
                last[eng] = inst.ins
                return inst

            ones_f = const.tile([128, 1], F32, tag="ones_f")
            chain("v", nc.vector.memset(ones_f, 1.0))
            ones_col = const.tile([1, 128], F16, tag="ones_col")
            chain("v", nc.vector.tensor_copy(
                ones_col, ones_f[0:1, 0:1].to_broadcast([1, 128])))
            ones8 = const.tile([128, 2, 16], FP8, tag="ones8")
            chain("v", nc.vector.memset(ones8, 1.0))
            ones_r = const.tile([128, 1], mybir.dt.float32r, tag="ones_r")
            chain("v", nc.vector.tensor_copy(ones_r, ones_f))
            zbias = const.tile([128, 1], F32, tag="zbias")
            zb_inst = chain("v", nc.vector.memset(zbias, 0.0))
            mbias = const.tile([128, 1], F32, tag="mbias")
            chain("v", nc.vector.memset(mbias, EXP_SHIFT))
            lbias = const.tile([128, 1], F32, tag="lbias")
            chain("v", nc.vector.memset(lbias, math.log(F_DIV / W_SCALE)))

            scr_act = const.tile([1, 1], F32, tag="scr_act")
            acol = nc.scalar.activation(
                scr_act, zbias[0:1, :], mybir.ActivationFunctionType.Copy
            )
            add_dep_helper(acol.ins, zb_inst.ins, True, "act bias barrier")
            last["a"] = acol.ins

            for k, ld in enumerate([b_load, xq_load]):
                scr_k = const.tile([1, 1], F32, tag=f"scr{k}", name=f"scr{k}")
                dcol = nc.vector.memset(scr_k, 0.0)
                add_dep_helper(dcol.ins, ld.ins, True, "dve input barrier")
                chain("v", dcol)

            ps_col = pso.tile([1, 1], F32, tag="po", name="ps_col")

            # PE clock-gate warmup: the HAM throttles a cold PE to 1.2 GHz
            # until it has seen ~3.4us of sustained activity. Dummy 1x1
            # matmuls during the input DMAs put the array at 2.4 GHz by the
            # time the first projection runs.
            wprobe = zbias[0:1, 0:1]
            for _ in range(32):
                chain("p", nc.tensor.matmul(ps_col, lhsT=wprobe, rhs=wprobe))

            probe = bb[0:1, 0, 0:1]

            def pe_barrier(ld):
                col = nc.tensor.matmul(ps_col, lhsT=probe, rhs=probe)
                add_dep_helper(col.ins, ld.ins, True, "pe input barrier")
                chain("p", col)

            pe_barrier(w_load)

            def mm(*args, **kwargs):
                return chain("p", nc.tensor.matmul(*args, **kwargs))

            def dve(fn, *args, **kwargs):
                return chain("v", fn(*args, **kwargs))

            def act(*args, **kwargs):
                return chain("a", nc.scalar.activation(*args, **kwargs))

            # ---- projections: one DR matmul per 512-col group. Pairs of
            # groups rotate through FOUR PSUM buffers (the two "s" slots
            # plus the not-yet-used f0/f1 banks), and each pair's eviction
            # (x1/16 weight-prescale compensation + bias) is split into two
            # 512 halves issued to ACT and DVE in parallel, so eviction
            # latency stays off the PE critical path.
    
            pair_idx = [0]
            s_only = [False]

            def project_pair(w_idx, dst, bias_col, gp, co):
                if s_only[0] or pair_idx[0] % 2 == 0:
                    ps_pj = pss.tile(
                        [128, 2, 512], F32, tag="s", bufs=2, name="ps_pj"
                    )
                    halves = [ps_pj[:, 0], ps_pj[:, 1]]
                else:
                    halves = [
                        psf.tile([128, 512], F32, tag=f"f{i}", name="ps_pj")
                        for i in range(2)
                    ]
                for g in (2 * gp, 2 * gp + 1):
                    mm(
                        halves[g % 2],
                        lhsT=ws_sb[:, w_idx, :, co * 128 : (co + 1) * 128],
                        rhs=xk_sb[
                            :, g // 2, :,
                            (g % 2) * 512 : (g % 2) * 512 + 512,
                        ],
                        perf_mode=DR,
                    )
                for h in range(2):
                    dsl = dst[
                        :, co,
                        gp * 1024 + h * 512 : gp * 1024 + (h + 1) * 512,
                    ]
                    if (h + pair_idx[0]) % 2:
                        dve(
                            nc.vector.tensor_scalar,
                            dsl, halves[h],
                            1.0 / W_SCALE,
                            bb[:, bias_col, co : co + 1],
                            op0=mult, op1=add,
                        )
                    else:
                        act(
                            dsl, halves[h],
                            mybir.ActivationFunctionType.Identity,
                            bias=bb[:, bias_col, co : co + 1],
                            scale=1.0 / W_SCALE,
                        )
                pair_idx[0] += 1

            # Only theta/phi group 0 (xk chunk 0) run before the attention
            # loop; the remaining projection pairs interleave into the first
            # attention groups, right before the score sweep reaches their
            # key columns (the xk chunks are still streaming in).
            pe_barrier(xk_loads[0])
            project_pair(0, th_sb, 0, 0, 0)
            project_pair(0, th_sb, 0, 0, 1)
            project_pair(1, ph_sb, 1, 0, 0)
            project_pair(1, ph_sb, 1, 0, 1)
            pe_barrier(xk_loads[1])
            project_pair(0, th_sb, 0, 1, 0)
            project_pair(0, th_sb, 0, 1, 1)
            pe_barrier(xk_loads[3])
            project_pair(1, ph_sb, 1, 3, 0)
            project_pair(1, ph_sb, 1, 3, 1)
            s_only[0] = True
            barriers_todo = {(0, 5): xk_loads[2]}
            proj_todo = {
                (0, 1): (1, ph_sb, 1, 1, 0),
                (0, 2): (1, ph_sb, 1, 1, 1),
                (0, 5): (1, ph_sb, 1, 2, 0),
                (0, 6): (1, ph_sb, 1, 2, 1),
            }
            xt_barrier = [xt_load]

            # ---- attention. The softmax normalization is applied AFTER the
            # conv (it commutes with the channel mixing), so the ps_f PSUM
            # accumulators are released by a plain fixed-scale eviction the
            # moment the group's PV finishes -- nothing on the PE ever waits
            # for the reciprocal chain. Each group's finalization steps are
            # spread through the NEXT group's qp pipeline:
            #   qp0: evict f (DVE, frees psf banks for this group's PV)
            #   qp1: ln + exp reciprocal (ACT, after this group's first exp)
            #   qp2: bc row-broadcast matmul + copy to SBUF
            #   qp3: conv co=0 ; qp4: normalize+residual+store co=0
            #   qp5: conv co=1 ; qp6: normalize+residual+store co=1
            deferred = [None]
            den_ring = pso.tile([1, 512], F32, tag="den", name="den_ring")

            def den_slot(pg):
                return den_ring

            def fin_evict(pg):
                psl = slice(pg * 512, (pg + 1) * 512)
                ps_fs = deferred[0][1]
                for ci in range(CT):
                    if pg == PG - 1 and ci == 0:
                        # tail latency: ACT is idle after the last exp, so
                        # the two evictions run on different engines
                        act(
                            f_sb[:, ci, psl], ps_fs[ci],
                            mybir.ActivationFunctionType.Identity,
                            scale=1.0 / F_DIV,
                        )
                    else:
                        dve(
                            nc.vector.tensor_scalar,
                            f_sb[:, ci, psl], ps_fs[ci],
                            1.0 / F_DIV, None, op0=mult,
                        )

            def fin_recip(pg):
                act(
                    lns[:, pg], den_slot(pg),
                    mybir.ActivationFunctionType.Ln,
                    bias=zbias[0:1],
                )
                act(
                    rc_ring[:, pg, :], lns[:, pg],
                    mybir.ActivationFunctionType.Exp,
                    bias=lbias[0:1], scale=-1.0,
                )

            def fin_bc(pg):
                ps_bc = pso.tile([128, 512], F32, tag="po", name="ps_bc")
                mm(ps_bc, lhsT=ones_col, rhs=rc_ring[:, pg, :])
                dve(nc.vector.tensor_copy, bc_ring[:, pg, :], ps_bc)

            conv_ps = {}

            def den_mm(ps_den, qpp):
                mm(
                    ps_den,
                    lhsT=ones8[:, :, 0:1],
                    rhs=e_ring[:, qpp % 4],
                    perf_mode=DR,
                    start=qpp == 0,
                    stop=False,
                )

            def den_dve(qpp):
                e_p = e_ring[:, qpp % 4]
                if qpp == 1:
                    dve(nc.vector.tensor_add, acc, e_p[:, 0], e_p[:, 1])
                else:
                    dve(nc.vector.tensor_add, tsum, e_p[:, 0], e_p[:, 1])
                    dve(nc.vector.tensor_add, acc, acc, tsum)

            def fin_conv(pg, co):
                psl = slice(pg * 512, (pg + 1) * 512)
                if pg == PG - 1 and co == 1:
                    ps_o = pss.tile(
                        [128, 2, 512], F32, tag="s", bufs=2, name="ps_o"
                    )[:, 0]
                else:
                    ps_o = pso.tile([128, 512], F32, tag="po", name="ps_o")
                mm(
                    ps_o,
                    lhsT=ws_sb[:, 2, :, co * 128 : (co + 1) * 128],
                    rhs=f_sb[:, :, psl],
                    perf_mode=DR,
                )
                conv_ps[co] = ps_o

            def fin_out(pg, co):
                psl = slice(pg * 512, (pg + 1) * 512)
                o_sl = o_ring[:, co, pg, :]
                dve(nc.vector.tensor_mul, o_sl, conv_ps.pop(co),
                    bc_ring[:, pg, :])
                dve(nc.vector.tensor_add, o_sl, o_sl, xq_sb[:, co, psl])
                nc.sync.dma_start(
                    out=out[co * 128 : (co + 1) * 128, psl], in_=o_sl
                )
                if co == CT - 1:
                    deferred[0] = None

            def fin_steps(qp, pg):
                if deferred[0] is None:
                    return
                prev = deferred[0][0]
                if qp == 0:
                    fin_evict(prev)
                elif qp == 1:
                    fin_recip(prev)
                elif qp == 2:
                    fin_bc(prev)
                elif qp == 3:
                    fin_conv(prev, 0)
                elif qp == 4:
                    fin_out(prev, 0)
                elif qp == 5:
                    fin_conv(prev, 1)
                elif qp == 6:
                    fin_out(prev, 1)

            for pg in range(PG):
                psl = slice(pg * 512, (pg + 1) * 512)
                ps_f = []
                ps_den = den_slot(pg)
                # software pipeline: PV runs one exp-pair behind scores so
                # the PE streams scores(k+1) while ACT computes exp(k)
                for qp in range(QT // 2 + 1):
                    if qp < QT // 2:
                        ps_s = pss.tile([128, 2, 512], F32, tag="s", bufs=2)
                        for sub in range(2):
                            qt = qp * 2 + sub
                            mm(
                                ps_s[:, sub],
                                lhsT=ph_sb[:, :, qt * 128 : (qt + 1) * 128],
                                rhs=th_sb[:, :, psl],
                                perf_mode=DR,
                            )
                        act(
                            e_ring[:, qp % 4], ps_s,
                            mybir.ActivationFunctionType.Exp,
                            bias=mbias, scale=scale,
                        )
                    if (pg, qp) in barriers_todo:
                        pe_barrier(barriers_todo.pop((pg, qp)))
                    if (pg, qp) in proj_todo:
                        project_pair(*proj_todo.pop((pg, qp)))
                    if qp == 1 and xt_barrier:
                        pe_barrier(xt_barrier.pop())
                    fin_steps(qp, pg)
                    if qp == 1:
                        # allocated AFTER fin_evict so the WAR against the
                        # previous group's eviction reads is registered
                        ps_f.extend(
                            psf.tile([128, 512], F32, tag=f"f{ci}",
                                     name=f"ps_f{ci}")
                            for ci in range(CT)
                        )
                    if qp >= 1:
                        qpp = qp - 1
                        e_p = e_ring[:, qpp % 4]
                        first, last_q = qpp == 0, qpp == QT // 2 - 1
                        for ci in range(CT):
                            mm(
                                ps_f[ci],
                                lhsT=xt_sb[
                                    :, 2 * qpp : 2 * qpp + 2,
                                    ci * 128 : (ci + 1) * 128,
                                ],
                                rhs=e_p,
                                perf_mode=DR,
                                start=first,
                                stop=last_q,
                            )
                    if qp >= 2 and (qp - 1) % 2 == 1:
                        # odd pairs: denominator partial sums on the Vector
                        # engine (it has idle capacity; the PE does not)
                        den_dve(qp - 1)
                    if qp >= 3 and (qp - 3) % 2 == 0:
                        # even pairs: ones-column matmul accumulated in PSUM.
                        # Issued 2 pairs behind PV so the WAR against the
                        # previous group's deferred ln read never stalls.
                        den_mm(ps_den, qp - 3)
                den_mm(ps_den, QT // 2 - 2)
                # fold the DVE partial into the PSUM accumulator and close it
                mm(ps_den, lhsT=ones_r, rhs=acc, start=False, stop=True)
                deferred[0] = (pg, ps_f)

            for qp in range(7):
                fin_steps(qp, PG)

    orig_gat = bacc.get_activation_tables
    bacc.get_activation_tables = _combined_ln_exp_tables(orig_gat)
    try:
        nc.compile()
    finally:
        bacc.get_activation_tables = orig_gat
    return nc


_PROGRAM_CACHE = {}


def _get_program(mm_dt=None):
    key = "fp8dr"
    if key not in _PROGRAM_CACHE:
        _PROGRAM_CACHE[key] = build_program(P_CORE, HW, C)
    return _PROGRAM_CACHE[key]


def make_in_maps(x, theta_w, theta_b, phi_w, phi_b, conv1_w, conv1_b,
                 mm_np=None):
    """Host-side sharding / layout prep (pure data movement + prescale).

    Every tensor is pre-shuffled into its on-chip SBUF layout
    (partition-major) so the device DMAs are large-descriptor copies.
    Channel c lives at (a, p) = (c // 128, c % 128) ... wait: c = a*128+p.
    """
    fp8 = mybir.dt.np(FP8)
    NCHUNK, QC, CT = 4, HW // 4, C // 128
    # [3, C, C] -> [128, 3, CT, C]  (w, (a p), o -> p, w, a, o)
    wcat = np.ascontiguousarray(
        np.clip(
            W_SCALE * np.stack(
                [
                    np.asarray(theta_w, np.float32).T,
                    np.asarray(phi_w, np.float32).T,
                    np.asarray(conv1_w, np.float32).T,
                ]
            ),
            -240.0, 240.0,
        ).reshape(3, CT, 128, C).transpose(2, 0, 1, 3).astype(fp8)
    )
    bcat = np.ascontiguousarray(
        np.stack(
            [
                np.asarray(theta_b, np.float32),
                np.asarray(phi_b, np.float32),
                np.asarray(conv1_b, np.float32),
            ]
        ).reshape(3, CT, 128).transpose(2, 0, 1)
    )
    xf = np.asarray(x, np.float32).reshape(N, C, HW)
    cb = np.asarray(conv1_b, np.float32)[:, None]
    in_maps = []
    for core in range(NCORES):
        n, half = divmod(core, CORES_PER_N)
        off = half * P_CORE
        xk_i = np.ascontiguousarray(np.roll(xf[n], -off, axis=1))
        # [C, HW] -> [NCHUNK, 128, CT, QC]
        xk8 = xk_i.astype(fp8).reshape(CT, 128, NCHUNK, QC)
        xk8 = np.ascontiguousarray(xk8.transpose(2, 1, 0, 3))
        # [C, P] -> [128, CT, P]
        xq16 = (xk_i[:, :P_CORE] + cb).astype(np.float16)
        xq16 = np.ascontiguousarray(
            xq16.reshape(CT, 128, P_CORE).transpose(1, 0, 2))
        # [HW, C] -> [128, HW//128, C]
        xt8 = np.ascontiguousarray(xk_i.T).astype(fp8)
        xt8 = np.ascontiguousarray(
            xt8.reshape(HW // 128, 128, C).transpose(1, 0, 2))
        in_maps.append(
            {
                "xk": xk8,
                "xqb": xq16,
                "xt": xt8,
                "wcat": wcat,
                "bcat": bcat,
            }
        )
    return in_maps


def assemble_output(results):
    y = np.empty((N, C, HW), np.float32)
    for core in range(NCORES):
        n, half = divmod(core, CORES_PER_N)
        off = half * P_CORE
        y[n][:, off : off + P_CORE] = results[core]["out"].astype(np.float32)
    return y.reshape(N, C, H, W)


def kernel(x, theta_w, theta_b, phi_w, phi_b, conv1_w, conv1_b,
           mm_dt=None, **run_kwargs):
    nc = _get_program()
    in_maps = make_in_maps(
        x, theta_w, theta_b, phi_w, phi_b, conv1_w, conv1_b
    )
    res = run_bass_kernel_spmd(nc, in_maps, list(range(NCORES)), **run_kwargs)
    out = assemble_output(res.results)
    kernel.last_results = res
    return out


# revision 39
# speedup vs baseline: 1.0201x; 1.0201x over previous
"""Trainium2 Bass kernel for nn_DenoisingModule (non-local attention block).

Reference computation (per batch element n, with C=256 channels, HW=4096):
    theta = W_t x + b_t            # queries  [C, HW]
    phi   = W_p x + b_p            # keys     [C, HW]
    g     = x                      # values   [C, HW]
    S     = theta^T phi / sqrt(C)  # [HW, HW]
    A     = softmax(S, axis=keys)
    f     = g A^T                  # [C, HW]
    out   = x + W_c f + b_c

Sharding: 8 cores; each of the N=4 batch elements is split across 2 cores
by query position (2048 queries per core). Every core holds the full key
set for its batch element, so no collectives are needed.

Per-core device program (SPMD, identical on all cores, data differs):
  - ALL matmuls run in fp8e4 DoubleRow mode (2 contraction k-tiles per
    instruction, 2 fp8 MACs/cell/cycle): the 256-channel contractions
    (projections, scores, conv) are one DR matmul each; the 4096-key PV
    contraction is 16 DR matmuls per 512-query group.
  - scores are computed TRANSPOSED (S^T[k, q] = phi^T theta) so the exp
    output E^T feeds the PV matmul directly as the fp8 moving operand.
  - softmax uses exp(s - 3) (constant shift, softmax-invariant) so the
    max exp value ~96 stays under the TRN fp8e4 max of 240.
  - the softmax denominator is split between engines to balance per-qp
    budgets: even exp pairs are a ones-column DR matmul accumulated in
    PSUM (PE), odd pairs are pairwise adds into an f32r accumulator
    (DVE), folded together by one ones-row matmul per group.
  - softmax normalization is applied AFTER the conv (it commutes with
    the channel mixing): the PV accumulators are released by a plain
    1/64 eviction the moment a group's PV finishes, so the PE never
    waits on the reciprocal (ln/exp on ACT) chain; the reciprocal is
    issued behind the NEXT group's first exp in ACT's in-order queue.
  - scale management for fp8: host prescales W by 16 (out of the fp8
    subnormal range), theta/phi evictions divide by 16; f is evicted as
    ps_f/64; the output stage multiplies by 4/sum (broadcast via a
    ones-column matmul) and adds the f16 residual (+bias host-folded).
  - the activation-table pass is patched so Ln and Exp share one table
    set (the default mapping reloads the ACT table RAMs per group).
  - projection evictions rotate through 4 PSUM buffers alternating
    between ACT and DVE; phi projections for key groups 1-2 interleave
    into attention group 0's pipeline while the xk chunks stream in.
  - input DMAs are issued in first-use order with xt/xqb deferred behind
    xk chunks, all host-pre-shuffled to partition-major >=2KB-contiguous
    layouts; outputs store per 512-query group as soon as they finalize.
  - dummy 1x1 matmuls during the input DMA warm the PE HAM clock gate
    (1.2 -> 2.4 GHz) before the first projection runs.

Toolchain constraint that shapes this file: every TPB engine instruction
(and every DMA) may carry at most ONE semaphore wait, so cross-engine
fan-in is funneled through per-engine collector chains, persistent ring
tiles replace rotating tile pools, and DMA-gated work sits behind
single-purpose barrier matmuls.

The host wrapper rolls x columns per-core so queries are always columns
[0, P) of the local key matrix (keeps the program identical across cores),
and pre-transposes x (and the weight matrices) since the PV matmul needs
x^T as the stationary operand.
"""

import math

import numpy as np

import concourse.bass as bass
import concourse.mybir as mybir
from concourse import bacc
from concourse.bass_utils import run_bass_kernel_spmd
from concourse.tile import TileContext, add_dep_helper


def _combined_ln_exp_tables(orig_fn):
    """Activation-table view that resolves Exp, Ln, Copy (and friends) to
    the single combined `natural_log_exp_and_others` set.

    The compiler's table-load pass picks the FIRST set containing each
    activation function; by default Exp resolves to `exp_and_others` and
    Ln to `natural_log`, so a kernel alternating exp and ln reloads the
    ACT table RAMs (~1.3us each) at every alternation. Hiding those
    functions from every other set makes all of this kernel's activations
    resolve to one set -> exactly one table load. Set ids keep their
    act_info.json positions, so the emitted act_func_set_id stays valid.
    """
    keep = "natural_log_exp_and_others"
    AFT = mybir.ActivationFunctionType
    hide = {AFT.Exp, AFT.Ln, AFT.Identity, AFT.Copy, AFT.MemsetZero}

    def patched(arch):
        tabs = orig_fn(arch)
        return {
            name: (funcs if name == keep else funcs - hide)
            for name, funcs in tabs.items()
        }

    return patched

N, C, H, W = 4, 256, 64, 64
HW = H * W
NCORES = 8
CORES_PER_N = NCORES // N
P_CORE = HW // CORES_PER_N  # queries per core

F32 = mybir.dt.float32
F16 = mybir.dt.float16
FP8 = mybir.dt.float8e4
DR = mybir.MatmulPerfMode.DoubleRow

EXP_SHIFT = -3.0  # softmax-invariant shift keeping exp under fp8e4 max 240
W_SCALE = 16.0    # host premultiplies weights (fp8 subnormal avoidance)
F_DIV = 64.0      # unnormalized f eviction divisor (fp8 range)


def build_program(P, Q, Cc=C):
    """Build the per-core Bass program.

    P: queries handled by this core (first P columns of xk)
    Q: total key positions
    """
    assert P % 512 == 0 and Q % 512 == 0 and Cc == 256
    CT = Cc // 128  # = 2: one DoubleRow pair
    QT = Q // 128
    PG = P // 512
    QG = Q // 512
    scale = float(Cc) ** -0.5

    # All inputs are pre-shuffled on the host into the SBUF layout
    # (partition-major, >=2KB contiguous per partition) so every load DMA
    # uses large descriptors instead of 256B-1KB strided runs.
    NCHUNK = 4
    QC = Q // NCHUNK
    CT_ = Cc // 128
    nc = bacc.Bacc("TRN2", target_bir_lowering=False)
    xk = nc.declare_dram_parameter(
        "xk", [NCHUNK, 128, CT_, QC], FP8, isOutput=False)[:]
    xqb = nc.declare_dram_parameter(
        "xqb", [128, CT_, P], F16, isOutput=False)[:]
    xt = nc.declare_dram_parameter(
        "xt", [128, Q // 128, Cc], FP8, isOutput=False)[:]
    wcat = nc.declare_dram_parameter(
        "wcat", [128, 3, CT_, Cc], FP8, isOutput=False)[:]
    bcat = nc.declare_dram_parameter(
        "bcat", [128, 3, CT_], F32, isOutput=False)[:]
    out = nc.declare_dram_parameter("out", [Cc, P], F16, isOutput=True)[:]

    add = mybir.AluOpType.add
    mult = mybir.AluOpType.mult

    with TileContext(nc) as tc:
        with (
            tc.tile_pool(name="const", bufs=1) as const,
            tc.tile_pool(name="big", bufs=1) as big,
            tc.tile_pool(name="pss", bufs=1, space="PSUM") as pss,
            tc.tile_pool(name="psf", bufs=1, space="PSUM") as psf,
            tc.tile_pool(name="pso", bufs=1, space="PSUM") as pso,
        ):
            QH = Q // 2

            # ---- input loads. Issue order matters: the sync queue drains
            # roughly in order, so the first projection's inputs (xk chunk 0
            # + weights) go first; xt/xqb are gated on the xk chunks so the
            # early transfers get full HBM bandwidth.
            xk_sb = big.tile([128, NCHUNK, CT, QC], FP8, tag="xk")

            def xk_load(h):
                return nc.sync.dma_start(out=xk_sb[:, h], in_=xk[h])

            xk_loads = [xk_load(0)]
            ws_sb = const.tile([128, 3, CT, Cc], FP8, tag="ws")
            w_load = nc.sync.dma_start(out=ws_sb, in_=wcat)
            bb = const.tile([128, 3, CT], F32, tag="bb")
            b_load = nc.sync.dma_start(out=bb, in_=bcat)
            xk_loads.append(xk_load(1))
            xt_sb = big.tile([128, QT, Cc], FP8, tag="xt")
            xt_load = nc.sync.dma_start(out=xt_sb, in_=xt)
            add_dep_helper(xt_load.ins, xk_loads[1].ins, True, "defer xt dma")
            xk_loads += [xk_load(h) for h in range(2, NCHUNK)]
            xq_sb = big.tile([128, CT, P], F16, tag="xq")
            xq_load = nc.sync.dma_start(out=xq_sb, in_=xqb)
            add_dep_helper(xq_load.ins, xk_loads[3].ins, True, "defer xqb dma")

            # persistent tiles (deliberately NOT pool-rotated: pool-slot
            # releases fan in multiple procs; rings keep wait fan-in low)
            acc = big.tile([128, 512], mybir.dt.float32r, tag="acc")
            tsum = big.tile([128, 512], mybir.dt.float32r, tag="tsum")
            th_sb = big.tile([128, CT, P], FP8, tag="th")
            ph_sb = big.tile([128, CT, Q], FP8, tag="ph")
            f_sb = big.tile([128, CT, P], FP8, tag="f")
            e_ring = big.tile([128, 4, 2, 512], FP8, tag="ering")
            rc_ring = const.tile([1, PG, 512], F16, tag="rcring")
            lns = const.tile([1, PG, 512], F32, tag="lns")
            bc_ring = big.tile([128, PG, 512], F32, tag="bcring")
            o_ring = big.tile([128, CT, PG, 512], F16, tag="oring")

            # ---- engine program-order chains + wait collectors ----
            last = {}

            def chain(eng, inst):
                # ordering edges disabled: Bacc legalizes multi-waits, so the
                # Tile scheduler is free to interleave within each engine
                last[eng] = inst.ins
                return inst

            ones_f = const.tile([128, 1], F32, tag="ones_f")
            chain("v", nc.vector.memset(ones_f, 1.0))
            ones_col = const.tile([1, 128], F16, tag="ones_col")
            chain("v", nc.vector.tensor_copy(
                ones_col, ones_f[0:1, 0:1].to_broadcast([1, 128])))
            ones8 = const.tile([128, 2, 16], FP8, tag="ones8")
            chain("v", nc.vector.memset(ones8, 1.0))
            ones_r = const.tile([128, 1], mybir.dt.float32r, tag="ones_r")
            chain("v", nc.vector.tensor_copy(ones_r, ones_f))
            zbias = const.tile([128, 1], F32, tag="zbias")
            zb_inst = chain("v", nc.vector.memset(zbias, 0.0))
            mbias = const.tile([128, 1], F32, tag="mbias")
            chain("v", nc.vector.memset(mbias, EXP_SHIFT))
            lbias = const.tile([128, 1], F32, tag="lbias")
            chain("v", nc.vector.memset(lbias, math.log(F_DIV / W_SCALE)))

            scr_act = const.tile([1, 1], F32, tag="scr_act")
            acol = nc.scalar.activation(
                scr_act, zbias[0:1, :], mybir.ActivationFunctionType.Copy
            )
            add_dep_helper(acol.ins, zb_inst.ins, True, "act bias barrier")
            last["a"] = acol.ins

            for k, ld in enumerate([b_load, xq_load]):
                scr_k = const.tile([1, 1], F32, tag=f"scr{k}", name=f"scr{k}")
                dcol = nc.vector.memset(scr_k, 0.0)
                add_dep_helper(dcol.ins, ld.ins, True, "dve input barrier")
                chain("v", dcol)

            ps_col = pso.tile([1, 1], F32, tag="po", name="ps_col")

            # PE clock-gate warmup: the HAM throttles a cold PE to 1.2 GHz
            # until it has seen ~3.4us of sustained activity. Dummy 1x1
            # matmuls during the input DMAs put the array at 2.4 GHz by the
            # time the first projection runs.
            wprobe = zbias[0:1, 0:1]
            for _ in range(44):
                chain("p", nc.tensor.matmul(ps_col, lhsT=wprobe, rhs=wprobe))

            probe = bb[0:1, 0, 0:1]

            def pe_barrier(ld):
                col = nc.tensor.matmul(ps_col, lhsT=probe, rhs=probe)
                add_dep_helper(col.ins, ld.ins, True, "pe input barrier")
                chain("p", col)

            pe_barrier(w_load)

            def mm(*args, **kwargs):
                return chain("p", nc.tensor.matmul(*args, **kwargs))

            def dve(fn, *args, **kwargs):
                return chain("v", fn(*args, **kwargs))

            def act(*args, **kwargs):
                return chain("a", nc.scalar.activation(*args, **kwargs))

            # ---- projections: one DR matmul per 512-col group. Pairs of
            # groups rotate through FOUR PSUM buffers (the two "s" slots
            # plus the not-yet-used f0/f1 banks), and each pair's eviction
            # (x1/16 weight-prescale compensation + bias) is split into two
            # 512 halves issued to ACT and DVE in parallel, so eviction
            # latency stays off the PE critical path.
    
            pair_idx = [0]
            s_only = [False]

            def project_pair(w_idx, dst, bias_col, gp, co):
                if s_only[0] or pair_idx[0] % 2 == 0:
                    ps_pj = pss.tile(
                        [128, 2, 512], F32, tag="s", bufs=2, name="ps_pj"
                    )
                    halves = [ps_pj[:, 0], ps_pj[:, 1]]
                else:
                    halves = [
                        psf.tile([128, 512], F32, tag=f"f{i}", name="ps_pj")
                        for i in range(2)
                    ]
                for g in (2 * gp, 2 * gp + 1):
                    mm(
                        halves[g % 2],
                        lhsT=ws_sb[:, w_idx, :, co * 128 : (co + 1) * 128],
                        rhs=xk_sb[
                            :, g // 2, :,
                            (g % 2) * 512 : (g % 2) * 512 + 512,
                        ],
                        perf_mode=DR,
                    )
                for h in range(2):
                    dsl = dst[
                        :, co,
                        gp * 1024 + h * 512 : gp * 1024 + (h + 1) * 512,
                    ]
                    if s_only[0] or (h + pair_idx[0]) % 2:
                        dve(
                            nc.vector.tensor_scalar,
                            dsl, halves[h],
                            1.0 / W_SCALE,
                            bb[:, bias_col, co : co + 1],
                            op0=mult, op1=add,
                        )
                    else:
                        act(
                            dsl, halves[h],
                            mybir.ActivationFunctionType.Identity,
                            bias=bb[:, bias_col, co : co + 1],
                            scale=1.0 / W_SCALE,
                        )
                pair_idx[0] += 1

            # Only theta/phi group 0 (xk chunk 0) run before the attention
            # loop; the remaining projection pairs interleave into the first
            # attention groups, right before the score sweep reaches their
            # key columns (the xk chunks are still streaming in).
            pe_barrier(xk_loads[0])
            project_pair(0, th_sb, 0, 0, 0)
            project_pair(0, th_sb, 0, 0, 1)
            project_pair(1, ph_sb, 1, 0, 0)
            project_pair(1, ph_sb, 1, 0, 1)
            pe_barrier(xk_loads[1])
            project_pair(0, th_sb, 0, 1, 0)
            project_pair(0, th_sb, 0, 1, 1)
            pe_barrier(xk_loads[3])
            project_pair(1, ph_sb, 1, 3, 0)
            project_pair(1, ph_sb, 1, 3, 1)
            s_only[0] = True
            barriers_todo = {(0, 5): xk_loads[2]}
            proj_todo = {
                (0, 1): (1, ph_sb, 1, 1, 0),
                (0, 2): (1, ph_sb, 1, 1, 1),
                (0, 5): (1, ph_sb, 1, 2, 0),
                (0, 6): (1, ph_sb, 1, 2, 1),
            }
            xt_barrier = [xt_load]

            # ---- attention. The softmax normalization is applied AFTER the
            # conv (it commutes with the channel mixing), so the ps_f PSUM
            # accumulators are released by a plain fixed-scale eviction the
            # moment the group's PV finishes -- nothing on the PE ever waits
            # for the reciprocal chain. Each group's finalization steps are
            # spread through the NEXT group's qp pipeline:
            #   qp0: evict f (DVE, frees psf banks for this group's PV)
            #   qp1: ln + exp reciprocal (ACT, after this group's first exp)
            #   qp2: bc row-broadcast matmul + copy to SBUF
            #   qp3: conv co=0 ; qp4: normalize+residual+store co=0
            #   qp5: conv co=1 ; qp6: normalize+residual+store co=1
            deferred = [None]
            den_ring = pso.tile([1, 512], F32, tag="den", name="den_ring")

            def den_slot(pg):
                return den_ring

            def fin_evict(pg):
                psl = slice(pg * 512, (pg + 1) * 512)
                ps_fs = deferred[0][1]
                for ci in range(CT):
                    if pg == PG - 1 and ci == 0:
                        # tail latency: ACT is idle after the last exp, so
                        # the two evictions run on different engines
                        act(
                            f_sb[:, ci, psl], ps_fs[ci],
                            mybir.ActivationFunctionType.Identity,
                            scale=1.0 / F_DIV,
                        )
                    else:
                        dve(
                            nc.vector.tensor_scalar,
                            f_sb[:, ci, psl], ps_fs[ci],
                            1.0 / F_DIV, None, op0=mult,
                        )

            def fin_recip(pg):
                act(
                    lns[:, pg], den_slot(pg),
                    mybir.ActivationFunctionType.Ln,
                    bias=zbias[0:1],
                )
                act(
                    rc_ring[:, pg, :], lns[:, pg],
                    mybir.ActivationFunctionType.Exp,
                    bias=lbias[0:1], scale=-1.0,
                )

            def fin_bc(pg):
                ps_bc = pso.tile([128, 512], F32, tag="po", name="ps_bc")
                mm(ps_bc, lhsT=ones_col, rhs=rc_ring[:, pg, :])
                dve(nc.vector.tensor_copy, bc_ring[:, pg, :], ps_bc)

            conv_ps = {}

            def den_mm(ps_den, qpp):
                mm(
                    ps_den,
                    lhsT=ones8[:, :, 0:1],
                    rhs=e_ring[:, qpp % 4],
                    perf_mode=DR,
                    start=qpp == 0,
                    stop=False,
                )

            def den_dve(qpp):
                e_p = e_ring[:, qpp % 4]
                if qpp == 1:
                    dve(nc.vector.tensor_add, acc, e_p[:, 0], e_p[:, 1])
                else:
                    dve(nc.vector.tensor_add, tsum, e_p[:, 0], e_p[:, 1])
                    dve(nc.vector.tensor_add, acc, acc, tsum)

            def fin_conv(pg, co):
                psl = slice(pg * 512, (pg + 1) * 512)
                if pg == PG - 1 and co == 1:
                    ps_o = pss.tile(
                        [128, 2, 512], F32, tag="s", bufs=2, name="ps_o"
                    )[:, 0]
                else:
                    ps_o = pso.tile([128, 512], F32, tag="po", name="ps_o")
                mm(
                    ps_o,
                    lhsT=ws_sb[:, 2, :, co * 128 : (co + 1) * 128],
                    rhs=f_sb[:, :, psl],
                    perf_mode=DR,
                )
                conv_ps[co] = ps_o

            def fin_out(pg, co):
                psl = slice(pg * 512, (pg + 1) * 512)
                o_sl = o_ring[:, co, pg, :]
                dve(nc.vector.tensor_mul, o_sl, conv_ps.pop(co),
                    bc_ring[:, pg, :])
                dve(nc.vector.tensor_add, o_sl, o_sl, xq_sb[:, co, psl])
                nc.sync.dma_start(
                    out=out[co * 128 : (co + 1) * 128, psl], in_=o_sl
                )
                if co == CT - 1:
                    deferred[0] = None

            def fin_steps(qp, pg):
                if deferred[0] is None:
                    return
                prev = deferred[0][0]
                if qp == 0:
                    fin_evict(prev)
                elif qp == 1:
                    fin_recip(prev)
                elif qp == 2:
                    fin_bc(prev)
                elif qp == 3:
                    fin_conv(prev, 0)
                elif qp == 4:
                    fin_out(prev, 0)
                elif qp == 5:
                    fin_conv(prev, 1)
                elif qp == 6:
                    fin_out(prev, 1)

            for pg in range(PG):
                psl = slice(pg * 512, (pg + 1) * 512)
                ps_f = []
                ps_den = den_slot(pg)
                # software pipeline: PV runs one exp-pair behind scores so
                # the PE streams scores(k+1) while ACT computes exp(k)
                for qp in range(QT // 2 + 1):
                    if qp < QT // 2:
                        ps_s = pss.tile([128, 2, 512], F32, tag="s", bufs=2)
                        for sub in range(2):
                            qt = qp * 2 + sub
                            mm(
                                ps_s[:, sub],
                                lhsT=ph_sb[:, :, qt * 128 : (qt + 1) * 128],
                                rhs=th_sb[:, :, psl],
                                perf_mode=DR,
                            )
                        act(
                            e_ring[:, qp % 4], ps_s,
                            mybir.ActivationFunctionType.Exp,
                            bias=mbias, scale=scale,
                        )
                    if (pg, qp) in barriers_todo:
                        pe_barrier(barriers_todo.pop((pg, qp)))
                    if (pg, qp) in proj_todo:
                        project_pair(*proj_todo.pop((pg, qp)))
                    if qp == 1 and xt_barrier:
                        pe_barrier(xt_barrier.pop())
                    fin_steps(qp, pg)
                    if qp == 1:
                        # allocated AFTER fin_evict so the WAR against the
                        # previous group's eviction reads is registered
                        ps_f.extend(
                            psf.tile([128, 512], F32, tag=f"f{ci}",
                                     name=f"ps_f{ci}")
                            for ci in range(CT)
                        )
                    if qp >= 1:
                        qpp = qp - 1
                        e_p = e_ring[:, qpp % 4]
                        first, last_q = qpp == 0, qpp == QT // 2 - 1
                        for ci in range(CT):
                            mm(
                                ps_f[ci],
                                lhsT=xt_sb[
                                    :, 2 * qpp : 2 * qpp + 2,
                                    ci * 128 : (ci + 1) * 128,
                                ],
                                rhs=e_p,
                                perf_mode=DR,
                                start=first,
                                stop=last_q,
                            )
                    if qp >= 2 and (qp - 1) % 2 == 1:
                        # odd pairs: denominator partial sums on the Vector
                        # engine (it has idle capacity; the PE does not)
                        den_dve(qp - 1)
                    if qp >= 3 and (qp - 3) % 2 == 0:
                        # even pairs: ones-column matmul accumulated in PSUM.
                        # Issued 2 pairs behind PV so the WAR against the
                        # previous group's deferred ln read never stalls.
                        den_mm(ps_den, qp - 3)
                den_mm(ps_den, QT // 2 - 2)
                # fold the DVE partial into the PSUM accumulator and close it
                mm(ps_den, lhsT=ones_r, rhs=acc, start=False, stop=True)
                deferred[0] = (pg, ps_f)

            for qp in range(7):
                fin_steps(qp, PG)

    orig_gat = bacc.get_activation_tables
    bacc.get_activation_tables = _combined_ln_exp_tables(orig_gat)
    try:
        nc.compile()
    finally:
        bacc.get_activation_tables = orig_gat
    return nc


_PROGRAM_CACHE = {}


def _get_program(mm_dt=None):
    key = "fp8dr"
    if key not in _PROGRAM_CACHE:
        _PROGRAM_CACHE[key] = build_program(P_CORE, HW, C)
    return _PROGRAM_CACHE[key]


def make_in_maps(x, theta_w, theta_b, phi_w, phi_b, conv1_w, conv1_b,
                 mm_np=None):
    """Host-side sharding / layout prep (pure data movement + prescale).

    Every tensor is pre-shuffled into its on-chip SBUF layout
    (partition-major) so the device DMAs are large-descriptor copies.
    Channel c lives at (a, p) = (c // 128, c % 128) ... wait: c = a*128+p.
    """
    fp8 = mybir.dt.np(FP8)
    NCHUNK, QC, CT = 4, HW // 4, C // 128
    # [3, C, C] -> [128, 3, CT, C]  (w, (a p), o -> p, w, a, o)
    wcat = np.ascontiguousarray(
        np.clip(
            W_SCALE * np.stack(
                [
                    np.asarray(theta_w, np.float32).T,
                    np.asarray(phi_w, np.float32).T,
                    np.asarray(conv1_w, np.float32).T,
                ]
            ),
            -240.0, 240.0,
        ).reshape(3, CT, 128, C).transpose(2, 0, 1, 3).astype(fp8)
    )
    bcat = np.ascontiguousarray(
        np.stack(
            [
                np.asarray(theta_b, np.float32),
                np.asarray(phi_b, np.float32),
                np.asarray(conv1_b, np.float32),
            ]
        ).reshape(3, CT, 128).transpose(2, 0, 1)
    )
    xf = np.asarray(x, np.float32).reshape(N, C, HW)
    cb = np.asarray(conv1_b, np.float32)[:, None]
    in_maps = []
    for core in range(NCORES):
        n, half = divmod(core, CORES_PER_N)
        off = half * P_CORE
        xk_i = np.ascontiguousarray(np.roll(xf[n], -off, axis=1))
        # [C, HW] -> [NCHUNK, 128, CT, QC]
        xk8 = xk_i.astype(fp8).reshape(CT, 128, NCHUNK, QC)
        xk8 = np.ascontiguousarray(xk8.transpose(2, 1, 0, 3))
        # [C, P] -> [128, CT, P]
        xq16 = (xk_i[:, :P_CORE] + cb).astype(np.float16)
        xq16 = np.ascontiguousarray(
            xq16.reshape(CT, 128, P_CORE).transpose(1, 0, 2))
        # [HW, C] -> [128, HW//128, C]
        xt8 = np.ascontiguousarray(xk_i.T).astype(fp8)
        xt8 = np.ascontiguousarray(
            xt8.reshape(HW // 128, 128, C).transpose(1, 0, 2))
        in_maps.append(
            {
                "xk": xk8,
                "xqb": xq16,
                "xt": xt8,
                "wcat": wcat,
                "bcat": bcat,
            }
        )
    return in_maps


def assemble_output(results):
    y = np.empty((N, C, HW), np.float32)
    for core in range(NCORES):
        n, half = divmod(core, CORES_PER_N)
        off = half * P_CORE
        y[n][:, off : off + P_CORE] = results[core]["out"].astype(np.float32)
    return y.reshape(N, C, H, W)


def kernel(x, theta_w, theta_b, phi_w, phi_b, conv1_w, conv1_b,
           mm_dt=None, **run_kwargs):
    nc = _get_program()
    in_maps = make_in_maps(
        x, theta_w, theta_b, phi_w, phi_b, conv1_w, conv1_b
    )
    res = run_bass_kernel_spmd(nc, in_maps, list(range(NCORES)), **run_kwargs)
    out = assemble_output(res.results)
    kernel.last_results = res
    return out


# revision 40
# speedup vs baseline: 1.0244x; 1.0043x over previous
"""Trainium2 Bass kernel for nn_DenoisingModule (non-local attention block).

Reference computation (per batch element n, with C=256 channels, HW=4096):
    theta = W_t x + b_t            # queries  [C, HW]
    phi   = W_p x + b_p            # keys     [C, HW]
    g     = x                      # values   [C, HW]
    S     = theta^T phi / sqrt(C)  # [HW, HW]
    A     = softmax(S, axis=keys)
    f     = g A^T                  # [C, HW]
    out   = x + W_c f + b_c

Sharding: 8 cores; each of the N=4 batch elements is split across 2 cores
by query position (2048 queries per core). Every core holds the full key
set for its batch element, so no collectives are needed.

Per-core device program (SPMD, identical on all cores, data differs):
  - ALL matmuls run in fp8e4 DoubleRow mode (2 contraction k-tiles per
    instruction, 2 fp8 MACs/cell/cycle): the 256-channel contractions
    (projections, scores, conv) are one DR matmul each; the 4096-key PV
    contraction is 16 DR matmuls per 512-query group.
  - scores are computed TRANSPOSED (S^T[k, q] = phi^T theta) so the exp
    output E^T feeds the PV matmul directly as the fp8 moving operand.
  - softmax uses exp(s - 3) (constant shift, softmax-invariant) so the
    max exp value ~96 stays under the TRN fp8e4 max of 240.
  - the softmax denominator is split between engines to balance per-qp
    budgets: even exp pairs are a ones-column DR matmul accumulated in
    PSUM (PE), odd pairs are pairwise adds into an f32r accumulator
    (DVE), folded together by one ones-row matmul per group.
  - softmax normalization is applied AFTER the conv (it commutes with
    the channel mixing): the PV accumulators are released by a plain
    1/64 eviction the moment a group's PV finishes, so the PE never
    waits on the reciprocal (ln/exp on ACT) chain; the reciprocal is
    issued behind the NEXT group's first exp in ACT's in-order queue.
  - scale management for fp8: host prescales W by 16 (out of the fp8
    subnormal range), theta/phi evictions divide by 16; f is evicted as
    ps_f/64; the output stage multiplies by 4/sum (broadcast via a
    ones-column matmul) and adds the f16 residual (+bias host-folded).
  - the activation-table pass is patched so Ln and Exp share one table
    set (the default mapping reloads the ACT table RAMs per group).
  - projection evictions rotate through 4 PSUM buffers alternating
    between ACT and DVE; phi projections for key groups 1-2 interleave
    into attention group 0's pipeline while the xk chunks stream in.
  - input DMAs are issued in first-use order with xt/xqb deferred behind
    xk chunks, all host-pre-shuffled to partition-major >=2KB-contiguous
    layouts; outputs store per 512-query group as soon as they finalize.
  - dummy 1x1 matmuls during the input DMA warm the PE HAM clock gate
    (1.2 -> 2.4 GHz) before the first projection runs.

Toolchain constraint that shapes this file: every TPB engine instruction
(and every DMA) may carry at most ONE semaphore wait, so cross-engine
fan-in is funneled through per-engine collector chains, persistent ring
tiles replace rotating tile pools, and DMA-gated work sits behind
single-purpose barrier matmuls.

The host wrapper rolls x columns per-core so queries are always columns
[0, P) of the local key matrix (keeps the program identical across cores),
and pre-transposes x (and the weight matrices) since the PV matmul needs
x^T as the stationary operand.
"""

import math

import numpy as np

import concourse.bass as bass
import concourse.mybir as mybir
from concourse import bacc
from concourse.bass_utils import run_bass_kernel_spmd
from concourse.tile import TileContext, add_dep_helper


def _combined_ln_exp_tables(orig_fn):
    """Activation-table view that resolves Exp, Ln, Copy (and friends) to
    the single combined `natural_log_exp_and_others` set.

    The compiler's table-load pass picks the FIRST set containing each
    activation function; by default Exp resolves to `exp_and_others` and
    Ln to `natural_log`, so a kernel alternating exp and ln reloads the
    ACT table RAMs (~1.3us each) at every alternation. Hiding those
    functions from every other set makes all of this kernel's activations
    resolve to one set -> exactly one table load. Set ids keep their
    act_info.json positions, so the emitted act_func_set_id stays valid.
    """
    keep = "natural_log_exp_and_others"
    AFT = mybir.ActivationFunctionType
    hide = {AFT.Exp, AFT.Ln, AFT.Identity, AFT.Copy, AFT.MemsetZero}

    def patched(arch):
        tabs = orig_fn(arch)
        return {
            name: (funcs if name == keep else funcs - hide)
            for name, funcs in tabs.items()
        }

    return patched

N, C, H, W = 4, 256, 64, 64
HW = H * W
NCORES = 8
CORES_PER_N = NCORES // N
P_CORE = HW // CORES_PER_N  # queries per core

F32 = mybir.dt.float32
F16 = mybir.dt.float16
FP8 = mybir.dt.float8e4
DR = mybir.MatmulPerfMode.DoubleRow

EXP_SHIFT = -3.0  # softmax-invariant shift keeping exp under fp8e4 max 240
W_SCALE = 16.0    # host premultiplies weights (fp8 subnormal avoidance)
F_DIV = 64.0      # unnormalized f eviction divisor (fp8 range)


def build_program(P, Q, Cc=C):
    """Build the per-core Bass program.

    P: queries handled by this core (first P columns of xk)
    Q: total key positions
    """
    assert P % 512 == 0 and Q % 512 == 0 and Cc == 256
    CT = Cc // 128  # = 2: one DoubleRow pair
    QT = Q // 128
    PG = P // 512
    QG = Q // 512
    scale = float(Cc) ** -0.5

    # All inputs are pre-shuffled on the host into the SBUF layout
    # (partition-major, >=2KB contiguous per partition) so every load DMA
    # uses large descriptors instead of 256B-1KB strided runs.
    NCHUNK = 4
    QC = Q // NCHUNK
    CT_ = Cc // 128
    nc = bacc.Bacc("TRN2", target_bir_lowering=False)
    xk = nc.declare_dram_parameter(
        "xk", [NCHUNK, 128, CT_, QC], FP8, isOutput=False)[:]
    xqb = nc.declare_dram_parameter(
        "xqb", [128, CT_, P], F16, isOutput=False)[:]
    xt = nc.declare_dram_parameter(
        "xt", [128, Q // 128, Cc], FP8, isOutput=False)[:]
    wcat = nc.declare_dram_parameter(
        "wcat", [128, 3, CT_, Cc], FP8, isOutput=False)[:]
    bcat = nc.declare_dram_parameter(
        "bcat", [128, 3, CT_], F32, isOutput=False)[:]
    out = nc.declare_dram_parameter("out", [Cc, P], F16, isOutput=True)[:]

    add = mybir.AluOpType.add
    mult = mybir.AluOpType.mult

    with TileContext(nc) as tc:
        with (
            tc.tile_pool(name="const", bufs=1) as const,
            tc.tile_pool(name="big", bufs=1) as big,
            tc.tile_pool(name="pss", bufs=1, space="PSUM") as pss,
            tc.tile_pool(name="psf", bufs=1, space="PSUM") as psf,
            tc.tile_pool(name="pso", bufs=1, space="PSUM") as pso,
        ):
            QH = Q // 2

            # ---- input loads. Issue order matters: the sync queue drains
            # roughly in order, so the first projection's inputs (xk chunk 0
            # + weights) go first; xt/xqb are gated on the xk chunks so the
            # early transfers get full HBM bandwidth.
            xk_sb = big.tile([128, NCHUNK, CT, QC], FP8, tag="xk")

            def xk_load(h):
                return nc.sync.dma_start(out=xk_sb[:, h], in_=xk[h])

            xk_loads = [xk_load(0)]
            ws_sb = const.tile([128, 3, CT, Cc], FP8, tag="ws")
            w_load = nc.sync.dma_start(out=ws_sb, in_=wcat)
            bb = const.tile([128, 3, CT], F32, tag="bb")
            b_load = nc.sync.dma_start(out=bb, in_=bcat)
            xk_loads.append(xk_load(1))
            xt_sb = big.tile([128, QT, Cc], FP8, tag="xt")
            xt_load = nc.sync.dma_start(out=xt_sb, in_=xt)
            add_dep_helper(xt_load.ins, xk_loads[1].ins, True, "defer xt dma")
            xk_loads += [xk_load(h) for h in range(2, NCHUNK)]
            xq_sb = big.tile([128, CT, P], F16, tag="xq")
            xq_load = nc.sync.dma_start(out=xq_sb, in_=xqb)
            add_dep_helper(xq_load.ins, xk_loads[3].ins, True, "defer xqb dma")

            # persistent tiles (deliberately NOT pool-rotated: pool-slot
            # releases fan in multiple procs; rings keep wait fan-in low)
            acc = big.tile([128, 512], mybir.dt.float32r, tag="acc")
            tsum = big.tile([128, 512], mybir.dt.float32r, tag="tsum")
            th_sb = big.tile([128, CT, P], FP8, tag="th")
            ph_sb = big.tile([128, CT, Q], FP8, tag="ph")
            f_sb = big.tile([128, CT, P], FP8, tag="f")
            e_ring = big.tile([128, 4, 2, 512], FP8, tag="ering")
            rc_ring = const.tile([1, PG, 512], F16, tag="rcring")
            lns = const.tile([1, PG, 512], F32, tag="lns")
            bc_ring = big.tile([128, PG, 512], F32, tag="bcring")
            o_ring = big.tile([128, CT, PG, 512], F16, tag="oring")

            # ---- engine program-order chains + wait collectors ----
            last = {}

            def chain(eng, inst):
                # ordering edges disabled: Bacc legalizes multi-waits, so the
                # Tile scheduler is free to interleave within each engine
                last[eng] = inst.ins
                return inst

            ones_f = const.tile([128, 1], F32, tag="ones_f")
            chain("v", nc.vector.memset(ones_f, 1.0))
            ones_col = const.tile([1, 128], F16, tag="ones_col")
            chain("v", nc.vector.tensor_copy(
                ones_col, ones_f[0:1, 0:1].to_broadcast([1, 128])))
            ones8 = const.tile([128, 2, 16], FP8, tag="ones8")
            chain("v", nc.vector.memset(ones8, 1.0))
            ones_r = const.tile([128, 1], mybir.dt.float32r, tag="ones_r")
            chain("v", nc.vector.tensor_copy(ones_r, ones_f))
            zbias = const.tile([128, 1], F32, tag="zbias")
            zb_inst = chain("v", nc.vector.memset(zbias, 0.0))
            mbias = const.tile([128, 1], F32, tag="mbias")
            chain("v", nc.vector.memset(mbias, EXP_SHIFT))
            lbias = const.tile([128, 1], F32, tag="lbias")
            chain("v", nc.vector.memset(lbias, math.log(F_DIV / W_SCALE)))

            scr_act = const.tile([1, 1], F32, tag="scr_act")
            acol = nc.scalar.activation(
                scr_act, zbias[0:1, :], mybir.ActivationFunctionType.Copy
            )
            add_dep_helper(acol.ins, zb_inst.ins, True, "act bias barrier")
            last["a"] = acol.ins

            for k, ld in enumerate([b_load, xq_load]):
                scr_k = const.tile([1, 1], F32, tag=f"scr{k}", name=f"scr{k}")
                dcol = nc.vector.memset(scr_k, 0.0)
                add_dep_helper(dcol.ins, ld.ins, True, "dve input barrier")
                chain("v", dcol)

            ps_col = pso.tile([1, 1], F32, tag="po", name="ps_col")

            # PE clock-gate warmup: the HAM throttles a cold PE to 1.2 GHz
            # until it has seen ~3.4us of sustained activity. Dummy 1x1
            # matmuls during the input DMAs put the array at 2.4 GHz by the
            # time the first projection runs.
            wprobe = zbias[0:1, 0:1]
            for _ in range(32):
                chain("p", nc.tensor.matmul(ps_col, lhsT=wprobe, rhs=wprobe))

            probe = bb[0:1, 0, 0:1]

            def pe_barrier(ld):
                col = nc.tensor.matmul(ps_col, lhsT=probe, rhs=probe)
                add_dep_helper(col.ins, ld.ins, True, "pe input barrier")
                chain("p", col)

            pe_barrier(w_load)

            def mm(*args, **kwargs):
                return chain("p", nc.tensor.matmul(*args, **kwargs))

            def dve(fn, *args, **kwargs):
                return chain("v", fn(*args, **kwargs))

            def act(*args, **kwargs):
                return chain("a", nc.scalar.activation(*args, **kwargs))

            # ---- projections: one DR matmul per 512-col group. Pairs of
            # groups rotate through FOUR PSUM buffers (the two "s" slots
            # plus the not-yet-used f0/f1 banks), and each pair's eviction
            # (x1/16 weight-prescale compensation + bias) is split into two
            # 512 halves issued to ACT and DVE in parallel, so eviction
            # latency stays off the PE critical path.
    
            pair_idx = [0]
            s_only = [False]

            def project_pair(w_idx, dst, bias_col, gp, co):
                if s_only[0] or pair_idx[0] % 2 == 0:
                    ps_pj = pss.tile(
                        [128, 2, 512], F32, tag="s", bufs=2, name="ps_pj"
                    )
                    halves = [ps_pj[:, 0], ps_pj[:, 1]]
                else:
                    halves = [
                        psf.tile([128, 512], F32, tag=f"f{i}", name="ps_pj")
                        for i in range(2)
                    ]
                for g in (2 * gp, 2 * gp + 1):
                    mm(
                        halves[g % 2],
                        lhsT=ws_sb[:, w_idx, :, co * 128 : (co + 1) * 128],
                        rhs=xk_sb[
                            :, g // 2, :,
                            (g % 2) * 512 : (g % 2) * 512 + 512,
                        ],
                        perf_mode=DR,
                    )
                for h in range(2):
                    dsl = dst[
                        :, co,
                        gp * 1024 + h * 512 : gp * 1024 + (h + 1) * 512,
                    ]
                    if (h + pair_idx[0]) % 2:
                        dve(
                            nc.vector.tensor_scalar,
                            dsl, halves[h],
                            1.0 / W_SCALE,
                            bb[:, bias_col, co : co + 1],
                            op0=mult, op1=add,
                        )
                    else:
                        act(
                            dsl, halves[h],
                            mybir.ActivationFunctionType.Identity,
                            bias=bb[:, bias_col, co : co + 1],
                            scale=1.0 / W_SCALE,
                        )
                pair_idx[0] += 1

            # Only theta/phi group 0 (xk chunk 0) run before the attention
            # loop; the remaining projection pairs interleave into the first
            # attention groups, right before the score sweep reaches their
            # key columns (the xk chunks are still streaming in).
            pe_barrier(xk_loads[0])
            project_pair(0, th_sb, 0, 0, 0)
            project_pair(0, th_sb, 0, 0, 1)
            project_pair(1, ph_sb, 1, 0, 0)
            project_pair(1, ph_sb, 1, 0, 1)
            pe_barrier(xk_loads[1])
            project_pair(0, th_sb, 0, 1, 0)
            project_pair(0, th_sb, 0, 1, 1)
            pe_barrier(xk_loads[3])
            project_pair(1, ph_sb, 1, 3, 0)
            project_pair(1, ph_sb, 1, 3, 1)
            s_only[0] = True
            barriers_todo = {(0, 5): xk_loads[2]}
            proj_todo = {
                (0, 1): (1, ph_sb, 1, 1, 0),
                (0, 2): (1, ph_sb, 1, 1, 1),
                (0, 5): (1, ph_sb, 1, 2, 0),
                (0, 6): (1, ph_sb, 1, 2, 1),
            }
            xt_barrier = [xt_load]

            # ---- attention. The softmax normalization is applied AFTER the
            # conv (it commutes with the channel mixing), so the ps_f PSUM
            # accumulators are released by a plain fixed-scale eviction the
            # moment the group's PV finishes -- nothing on the PE ever waits
            # for the reciprocal chain. Each group's finalization steps are
            # spread through the NEXT group's qp pipeline:
            #   qp0: evict f (DVE, frees psf banks for this group's PV)
            #   qp1: ln + exp reciprocal (ACT, after this group's first exp)
            #   qp2: bc row-broadcast matmul + copy to SBUF
            #   qp3: conv co=0 ; qp4: normalize+residual+store co=0
            #   qp5: conv co=1 ; qp6: normalize+residual+store co=1
            deferred = [None]
            den_ring = pso.tile([1, 512], F32, tag="den", name="den_ring")

            def den_slot(pg):
                return den_ring

            def fin_evict(pg):
                psl = slice(pg * 512, (pg + 1) * 512)
                ps_fs = deferred[0][1]
                for ci in range(CT):
                    if pg == PG - 1 and ci == 0:
                        # tail latency: ACT is idle after the last exp, so
                        # the two evictions run on different engines
                        act(
                            f_sb[:, ci, psl], ps_fs[ci],
                            mybir.ActivationFunctionType.Identity,
                            scale=1.0 / F_DIV,
                        )
                    else:
                        dve(
                            nc.vector.tensor_scalar,
                            f_sb[:, ci, psl], ps_fs[ci],
                            1.0 / F_DIV, None, op0=mult,
                        )

            def fin_recip(pg):
                act(
                    lns[:, pg], den_slot(pg),
                    mybir.ActivationFunctionType.Ln,
                    bias=zbias[0:1],
                )
                act(
                    rc_ring[:, pg, :], lns[:, pg],
                    mybir.ActivationFunctionType.Exp,
                    bias=lbias[0:1], scale=-1.0,
                )

            def fin_bc(pg):
                ps_bc = pso.tile([128, 512], F32, tag="po", name="ps_bc")
                mm(ps_bc, lhsT=ones_col, rhs=rc_ring[:, pg, :])
                dve(nc.vector.tensor_copy, bc_ring[:, pg, :], ps_bc)

            conv_ps = {}

            def den_mm(ps_den, qpp):
                mm(
                    ps_den,
                    lhsT=ones8[:, :, 0:1],
                    rhs=e_ring[:, qpp % 4],
                    perf_mode=DR,
                    start=qpp == 0,
                    stop=False,
                )

            def den_dve(qpp):
                e_p = e_ring[:, qpp % 4]
                if qpp == 1:
                    dve(nc.vector.tensor_add, acc, e_p[:, 0], e_p[:, 1])
                else:
                    dve(nc.vector.tensor_add, tsum, e_p[:, 0], e_p[:, 1])
                    dve(nc.vector.tensor_add, acc, acc, tsum)

            def fin_conv(pg, co):
                psl = slice(pg * 512, (pg + 1) * 512)
                if pg == PG - 1 and co == 1:
                    ps_o = pss.tile(
                        [128, 2, 512], F32, tag="s", bufs=2, name="ps_o"
                    )[:, 0]
                else:
                    ps_o = pso.tile([128, 512], F32, tag="po", name="ps_o")
                mm(
                    ps_o,
                    lhsT=ws_sb[:, 2, :, co * 128 : (co + 1) * 128],
                    rhs=f_sb[:, :, psl],
                    perf_mode=DR,
                )
                conv_ps[co] = ps_o

            def fin_out(pg, co):
                psl = slice(pg * 512, (pg + 1) * 512)
                o_sl = o_ring[:, co, pg, :]
                dve(nc.vector.tensor_mul, o_sl, conv_ps.pop(co),
                    bc_ring[:, pg, :])
                dve(nc.vector.tensor_add, o_sl, o_sl, xq_sb[:, co, psl])
                nc.sync.dma_start(
                    out=out[co * 128 : (co + 1) * 128, psl], in_=o_sl
                )
                if co == CT - 1:
                    deferred[0] = None

            def fin_steps(qp, pg):
                if deferred[0] is None:
                    return
                prev = deferred[0][0]
                if qp == 0:
                    fin_evict(prev)
                elif qp == 1:
                    fin_recip(prev)
                elif qp == 2:
                    fin_bc(prev)
                elif qp == 3:
                    fin_conv(prev, 0)
                elif qp == 4:
                    fin_out(prev, 0)
                elif qp == 5:
                    fin_conv(prev, 1)
                elif qp == 6:
                    fin_out(prev, 1)

            for pg in range(PG):
                psl = slice(pg * 512, (pg + 1) * 512)
                ps_f = []
                ps_den = den_slot(pg)
                # software pipeline: PV runs one exp-pair behind scores so
                # the PE streams scores(k+1) while ACT computes exp(k)
                for qp in range(QT // 2 + 1):
                    if qp < QT // 2:
                        ps_s = pss.tile([128, 2, 512], F32, tag="s", bufs=2)
                        for sub in range(2):
                            qt = qp * 2 + sub
                            mm(
                                ps_s[:, sub],
                                lhsT=ph_sb[:, :, qt * 128 : (qt + 1) * 128],
                                rhs=th_sb[:, :, psl],
                                perf_mode=DR,
                            )
                        act(
                            e_ring[:, qp % 4], ps_s,
                            mybir.ActivationFunctionType.Exp,
                            bias=mbias, scale=scale,
                        )
                    if (pg, qp) in barriers_todo:
                        pe_barrier(barriers_todo.pop((pg, qp)))
                    if (pg, qp) in proj_todo:
                        project_pair(*proj_todo.pop((pg, qp)))
                    if qp == 1 and xt_barrier:
                        pe_barrier(xt_barrier.pop())
                    fin_steps(qp, pg)
                    if qp == 1:
                        # allocated AFTER fin_evict so the WAR against the
                        # previous group's eviction reads is registered
                        ps_f.extend(
                            psf.tile([128, 512], F32, tag=f"f{ci}",
                                     name=f"ps_f{ci}")
                            for ci in range(CT)
                        )
                    if qp >= 1:
                        qpp = qp - 1
                        e_p = e_ring[:, qpp % 4]
                        first, last_q = qpp == 0, qpp == QT // 2 - 1
                        for ci in range(CT):
                            mm(
                                ps_f[ci],
                                lhsT=xt_sb[
                                    :, 2 * qpp : 2 * qpp + 2,
                                    ci * 128 : (ci + 1) * 128,
                                ],
                                rhs=e_p,
                                perf_mode=DR,
                                start=first,
                                stop=last_q,
                            )
                    if qp >= 2 and (qp - 1) % 2 == 1:
                        # odd pairs: denominator partial sums on the Vector
                        # engine (it has idle capacity; the PE does not)
                        den_dve(qp - 1)
                    if qp >= 3 and (qp - 3) % 2 == 0:
                        # even pairs: ones-column matmul accumulated in PSUM.
                        # Issued 2 pairs behind PV so the WAR against the
                        # previous group's deferred ln read never stalls.
                        den_mm(ps_den, qp - 3)
                den_mm(ps_den, QT // 2 - 2)
                # fold the DVE partial into the PSUM accumulator and close it
                mm(ps_den, lhsT=ones_r, rhs=acc, start=False, stop=True)
                deferred[0] = (pg, ps_f)

            for qp in range(7):
                fin_steps(qp, PG)

    orig_gat = bacc.get_activation_tables
    bacc.get_activation_tables = _combined_ln_exp_tables(orig_gat)
    try:
        nc.compile()
    finally:
        bacc.get_activation_tables = orig_gat
    return nc


_PROGRAM_CACHE = {}


def _get_program(mm_dt=None):
    key = "fp8dr"
    if key not in _PROGRAM_CACHE:
        _PROGRAM_CACHE[key] = build_program(P_CORE, HW, C)
    return _PROGRAM_CACHE[key]


def make_in_maps(x, theta_w, theta_b, phi_w, phi_b, conv1_w, conv1_b,
                 mm_np=None):
    """Host-side sharding / layout prep (pure data movement + prescale).

    Every tensor is pre-shuffled into its on-chip SBUF layout
    (partition-major) so the device DMAs are large-descriptor copies.
    Channel c lives at (a, p) = (c // 128, c % 128) ... wait: c = a*128+p.
    """
    fp8 = mybir.dt.np(FP8)
    NCHUNK, QC, CT = 4, HW // 4, C // 128
    # [3, C, C] -> [128, 3, CT, C]  (w, (a p), o -> p, w, a, o)
    wcat = np.ascontiguousarray(
        np.clip(
            W_SCALE * np.stack(
                [
                    np.asarray(theta_w, np.float32).T,
                    np.asarray(phi_w, np.float32).T,
                    np.asarray(conv1_w, np.float32).T,
                ]
            ),
            -240.0, 240.0,
        ).reshape(3, CT, 128, C).transpose(2, 0, 1, 3).astype(fp8)
    )
    bcat = np.ascontiguousarray(
        np.stack(
            [
                np.asarray(theta_b, np.float32),
                np.asarray(phi_b, np.float32),
                np.asarray(conv1_b, np.float32),
            ]
        ).reshape(3, CT, 128).transpose(2, 0, 1)
    )
    xf = np.asarray(x, np.float32).reshape(N, C, HW)
    cb = np.asarray(conv1_b, np.float32)[:, None]
    in_maps = []
    for core in range(NCORES):
        n, half = divmod(core, CORES_PER_N)
        off = half * P_CORE
        xk_i = np.ascontiguousarray(np.roll(xf[n], -off, axis=1))
        # [C, HW] -> [NCHUNK, 128, CT, QC]
        xk8 = xk_i.astype(fp8).reshape(CT, 128, NCHUNK, QC)
        xk8 = np.ascontiguousarray(xk8.transpose(2, 1, 0, 3))
        # [C, P] -> [128, CT, P]
        xq16 = (xk_i[:, :P_CORE] + cb).astype(np.float16)
        xq16 = np.ascontiguousarray(
            xq16.reshape(CT, 128, P_CORE).transpose(1, 0, 2))
        # [HW, C] -> [128, HW//128, C]
        xt8 = np.ascontiguousarray(xk_i.T).astype(fp8)
        xt8 = np.ascontiguousarray(
            xt8.reshape(HW // 128, 128, C).transpose(1, 0, 2))
        in_maps.append(
            {
                "xk": xk8,
                "xqb": xq16,
                "xt": xt8,
                "wcat": wcat,
                "bcat": bcat,
            }
        )
    return in_maps


def assemble_output(results):
    y = np.empty((N, C, HW), np.float32)
    for core in range(NCORES):
        n, half = divmod(core, CORES_PER_N)
        off = half * P_CORE
        y[n][:, off : off + P_CORE] = results[core]["out"].astype(np.float32)
    return y.reshape(N, C, H, W)


def kernel(x, theta_w, theta_b, phi_w, phi_b, conv1_w, conv1_b,
           mm_dt=None, **run_kwargs):
    nc = _get_program()
    in_maps = make_in_maps(
        x, theta_w, theta_b, phi_w, phi_b, conv1_w, conv1_b
    )
    res = run_bass_kernel_spmd(nc, in_maps, list(range(NCORES)), **run_kwargs)
    out = assemble_output(res.results)
    kernel.last_results = res
    return out
